# revision 22
# baseline (speedup 1.0000x reference)
"""Trainium2 Bass kernel for nn_Cov_EBFLayer.

Math: out[b,o] = exp(-quad[o,b]),
  quad[o,b] = diff^T P_o diff,  diff = c_o - x_b,  P_o = B_o B_o^T  (PSD Gram)
            = sum_{d,f} P[o,d,f] x_d x_f - 2 v_o^T x + q3_o,  v = P c, q3 = c^T P c

Kernel strategy (per core, batch-sharded 8 x 1024):
  Symmetric-pair feature map over cyclic offsets: unordered pairs {d, f} at
  cyclic distance k are covered once by offset-k rows (d, (d+k)%64), k=1..32.
  17 feature chunks of 128 rows x 1024 batch:
    - 16 "pair" chunks, offsets (2j+1, 2j+2) in the two 64-row halves.
      Built per-chunk via one of three paths (engine balancing):
        S: one full-width DVE tensor_mul of xb2=[x;x] against a rotated
           "slot" operand (host-precomputed rotations, loaded in one DMA)
        P: same, on the GPSIMD engine
        A: PE indicator matmul computes s = x_d + x_f into PSUM, ACT engine
           squares it: u = (x_d+x_f)^2; host adjusts W (A=coeff/2) and folds
           the unwanted x^2 cross terms into the diagonal weights.
    - 1 "misc" chunk: rows 0:64 = x_d^2 (DVE), rows 64:128 = x_d (DVE copy).
  Main contraction: per chunk, 4 accumulating matmuls (2 o-halves x 2 b-tiles
  of 512, one PSUM bank each). Epilogue: ACT Exp(scale=-1, bias=-q3) -> fp16.
Host does weight prep (P = beta beta^T, W chunk layout, v, q3; O(model)) and
layout-only data movement (x transpose + rotated copies).
"""

import sys
from contextlib import ExitStack

import numpy as np

sys.path.insert(0, "/opt/trn_rl_repo")

import concourse.bass as bass  # noqa: E402
import concourse.tile as tile  # noqa: E402
from concourse import bacc, mybir  # noqa: E402
from concourse import bass_utils  # noqa: E402
from concourse._compat import with_exitstack  # noqa: E402

B, D, O, NCORES = 8192, 64, 256, 8
BSH = B // NCORES  # 1024 per-core batch shard
BT = 512  # matmul free-dim tile (one PSUM bank of fp32)
F32 = mybir.dt.float32
F16 = mybir.dt.float16

# Accumulation-order chunk sequence. Pair chunk at list position j (skipping
# the misc entry) gets cyclic offsets (2j+1, 2j+2); paths: M=misc,
# S=slot DVE mul, P=slot GPSIMD mul, A=PE sum + ACT square.
_PATTERN = ["M", "A", "S", "A", "S", "A", "S", "A", "S", "A", "S", "P", "S", "A", "P", "S", "P"]
SEQ = []
_pj = 0
for _p in _PATTERN:
    if _p == "M":
        SEQ.append(("M", 0, 0))
    else:
        SEQ.append((_p, 2 * _pj + 1, 2 * _pj + 2))
        _pj += 1
NCH = len(SEQ)  # 17
NACT = sum(1 for s in SEQ if s[0] == "A")
SLOT_POS = [j for j, s in enumerate(SEQ) if s[0] in ("S", "P")]
NSLOT = len(SLOT_POS)
NSLOT_A = min(4, NSLOT)  # first slots arrive in an early DMA
W1CH = 4  # chunks in the first W transfer (so main matmuls start early)


@with_exitstack
def _kernel(ctx: ExitStack, tc, outT, xT, wts, ind, bias, xslots):
    nc = tc.nc

    cpool = ctx.enter_context(tc.tile_pool(name="const", bufs=1))
    gpool = ctx.enter_context(tc.tile_pool(name="gtiles", bufs=5))
    opool = ctx.enter_context(tc.tile_pool(name="outs", bufs=4))
    qpool = ctx.enter_context(tc.tile_pool(name="psum_q", bufs=4, space="PSUM"))
    spool = ctx.enter_context(tc.tile_pool(name="psum_s", bufs=4, space="PSUM"))

    # ---- resident inputs; order sets DMA-queue drain priority ----
    xb2 = cpool.tile([128, BSH], F16)  # [x; x] stacked
    nc.sync.dma_start(xb2[0:D, :], xT[:])
    i_sb = cpool.tile([D, NACT * 128], F16)
    nc.sync.dma_start(i_sb[:], ind[:])
    b_sb = cpool.tile([128, 2], F32)  # -q3 per o-half
    nc.sync.dma_start(b_sb[:], bias[:])
    w_sb = cpool.tile([128, NCH * O], F16)
    nc.sync.dma_start(w_sb[:, 0 : W1CH * O], wts[:, 0 : W1CH * O])
    nc.sync.dma_start(w_sb[:, W1CH * O :], wts[:, W1CH * O :])
    slots_sb = cpool.tile([128, NSLOT * BSH], F16)
    nc.sync.dma_start(
        slots_sb[:, 0 : NSLOT_A * BSH], xslots[:, 0 : NSLOT_A * BSH]
    )
    nc.sync.dma_start(slots_sb[:, NSLOT_A * BSH :], xslots[:, NSLOT_A * BSH :])

    # duplicate x rows on DVE (4x copy) instead of re-reading HBM
    g_misc = cpool.tile([128, BSH], F16)  # [x^2; x]
    nc.vector.tensor_copy(xb2[D : 2 * D, :], xb2[0:D, :])
    nc.vector.tensor_copy(g_misc[D:128, :], xb2[0:D, :])

    # PSUM bank = 2 KB/partition, so matmul outputs are [128, 512] fp32 max;
    # quad accumulates in 4 tiles (2 o-halves x 2 b-tiles).
    pq = {
        (oh, bt): qpool.tile([128, BT], F32, name=f"pq{oh}{bt}", tag="pq")
        for oh in range(2)
        for bt in range(2)
    }

    # ---- PE warm-up: a few matmuls to bridge the gap until the first G
    # chunk is ready (keeps HAM activity up). Overwritten by the real
    # accumulation. ----
    for i in range(4):
        nc.tensor.matmul(
            pq[(i % 2, (i // 2) % 2)][:],
            xb2[0:D, 0:128],
            xb2[0:D, 0:BT],
            start=True,
            stop=True,
        )

    # misc diag half: x_d^2
    nc.vector.tensor_mul(g_misc[0:D, :], xb2[0:D, :], xb2[0:D, :])

    act_pos = [j for j, s in enumerate(SEQ) if s[0] == "A"]
    slot_of = {j: si for si, j in enumerate(SLOT_POS)}
    s_tiles = {}
    state = {"ind_ptr": 0, "squares_done": 0}

    def top_up_inds():
        # keep <=2 chunks of indicator matmuls in flight ahead of the squares
        while (
            state["ind_ptr"] < len(act_pos)
            and state["ind_ptr"] - state["squares_done"] < 2
        ):
            ai = state["ind_ptr"]
            j = act_pos[ai]
            ss = []
            for bt in range(2):
                s = spool.tile([128, BT], F32, tag="s")
                nc.tensor.matmul(
                    s[:],
                    i_sb[:, ai * 128 : (ai + 1) * 128],
                    xb2[0:D, bt * BT : (bt + 1) * BT],
                    start=True,
                    stop=True,
                )
                ss.append(s)
            s_tiles[j] = ss
            state["ind_ptr"] += 1

    # ---- main loop: build G chunk, 4 accumulating matmuls ----
    for j, (p, k1, k2) in enumerate(SEQ):
        top_up_inds()
        if p == "M":
            g = g_misc
        elif p in ("S", "P"):
            g = gpool.tile([128, BSH], F16, tag="g")
            eng = nc.vector if p == "S" else nc.gpsimd
            si = slot_of[j]
            eng.tensor_mul(
                g[:], xb2[:], slots_sb[:, si * BSH : (si + 1) * BSH]
            )
        else:  # A: square the pair-sums on ACT straight out of PSUM
            g = gpool.tile([128, BSH], F16, tag="g")
            for bt in range(2):
                nc.scalar.activation(
                    g[:, bt * BT : (bt + 1) * BT],
                    s_tiles[j][bt][:],
                    mybir.ActivationFunctionType.Square,
                )
            state["squares_done"] += 1
        for bt in range(2):
            for oh in range(2):
                nc.tensor.matmul(
                    pq[(oh, bt)][:],
                    w_sb[:, j * O + oh * 128 : j * O + oh * 128 + 128],
                    g[:, bt * BT : (bt + 1) * BT],
                    start=(j == 0),
                    stop=(j == NCH - 1),
                )

    # ---- epilogue: out = exp(-(quad + q3)), one DMA per PSUM tile ----
    for oh in range(2):
        for bt in range(2):
            ob = opool.tile([128, BT], F16, tag="ob")
            nc.scalar.activation(
                ob[:],
                pq[(oh, bt)][:],
                mybir.ActivationFunctionType.Exp,
                bias=b_sb[:, oh : oh + 1],
                scale=-1.0,
            )
            nc.sync.dma_start(
                outT[oh * 128 : (oh + 1) * 128, bt * BT : (bt + 1) * BT], ob[:]
            )


_CACHE = {}


def _build():
    if "nc" in _CACHE:
        return _CACHE["nc"], _CACHE["aps"]
    nc = bacc.Bacc(
        "TRN2", target_bir_lowering=False, debug=False, num_devices=NCORES
    )
    xT = nc.dram_tensor("xT", [D, BSH], F16, kind="ExternalInput").ap()
    wts = nc.dram_tensor("wts", [128, NCH * O], F16, kind="ExternalInput").ap()
    ind = nc.dram_tensor("ind", [D, NACT * 128], F16, kind="ExternalInput").ap()
    bias = nc.dram_tensor("bias", [128, 2], F32, kind="ExternalInput").ap()
    xslots = nc.dram_tensor(
        "xslots", [128, NSLOT * BSH], F16, kind="ExternalInput"
    ).ap()
    outT = nc.dram_tensor("outT", [O, BSH], F16, kind="ExternalOutput").ap()
    with tile.TileContext(nc) as tc:
        _kernel(tc, outT, xT, wts, ind, bias, xslots)
    nc.compile()
    _CACHE["nc"] = nc
    _CACHE["aps"] = (xT, wts, ind, bias, xslots, outT)
    return nc, _CACHE["aps"]


def _host_prep(x, centers, betas):
    x32 = np.asarray(x, np.float32)
    betas32 = np.asarray(betas, np.float32)
    cen = np.asarray(centers, np.float32).reshape(O, D)
    # weight prep: O(model), batch-independent
    P = np.matmul(betas32, betas32.transpose(0, 2, 1))  # [O, D, D]
    w = np.einsum("ofe,of->oe", betas32, cen)
    v = np.einsum("ode,oe->od", betas32, w)
    q3 = np.einsum("oe,oe->o", w, w)

    dd = np.arange(D)
    R = np.zeros((O, D), np.float32)  # x^2 corrections from A-chunks
    Wstack = np.zeros((NCH, 128, O), np.float32)
    Istack = []
    for j, (p, k1, k2) in enumerate(SEQ):
        if p == "M":
            continue
        for half, k in ((0, k1), (1, k2)):
            f = (dd + k) % D
            coeff = (2.0 if k < D // 2 else 1.0) * P[:, dd, f]  # [O, 64]
            if p == "A":
                A_ = coeff * 0.5
                Wstack[j, half * D : (half + 1) * D, :] = A_.T
                R[:, dd] += A_
                R[:, f] += A_  # f is a permutation: indices unique
            else:
                Wstack[j, half * D : (half + 1) * D, :] = coeff.T
        if p == "A":
            I = np.zeros((D, 128), np.float32)
            pp = np.arange(128)
            dcol = pp % D
            kcol = np.where(pp < D, k1, k2)
            I[dcol, pp] += 1.0
            I[(dcol + kcol) % D, pp] += 1.0
            Istack.append(I)
    mj = next(j for j, s in enumerate(SEQ) if s[0] == "M")
    Wstack[mj, 0:D, :] = (P[:, dd, dd] - R).T
    Wstack[mj, D:128, :] = (-2.0 * v).T

    wts = np.ascontiguousarray(
        Wstack.transpose(1, 0, 2).reshape(128, NCH * O)
    ).astype(np.float16)
    ind = np.ascontiguousarray(np.concatenate(Istack, axis=1)).astype(np.float16)
    bias = np.ascontiguousarray((-q3).reshape(2, 128).T).astype(np.float32)

    xT_shards = []
    xslot_shards = []
    for i in range(NCORES):
        xTi = np.ascontiguousarray(
            x32[i * BSH : (i + 1) * BSH].T
        ).astype(np.float16)
        xT_shards.append(xTi)
        # rotated slot operands, in consumption order (layout-only gather)
        sl = np.empty((128, NSLOT, BSH), np.float16)
        for si, j in enumerate(SLOT_POS):
            _, k1, k2 = SEQ[j]
            sl[0:D, si, :] = np.roll(xTi, -k1, axis=0)
            sl[D:128, si, :] = np.roll(xTi, -k2, axis=0)
        xslot_shards.append(
            np.ascontiguousarray(sl.reshape(128, NSLOT * BSH))
        )
    return xT_shards, xslot_shards, wts, ind, bias


def _run(x, centers, betas, trace=False):
    nc, (xT, wts_ap, ind_ap, bias_ap, xslots_ap, outT) = _build()
    xT_shards, xslot_shards, wts, ind, bias = _host_prep(x, centers, betas)
    in_maps = [
        {
            xT.name: xT_shards[i],
            wts_ap.name: wts,
            ind_ap.name: ind,
            bias_ap.name: bias,
            xslots_ap.name: xslot_shards[i],
        }
        for i in range(NCORES)
    ]
    res = bass_utils.run_bass_kernel_spmd(
        nc, in_maps, core_ids=list(range(NCORES)), trace=trace
    )
    out = np.concatenate(
        [np.asarray(res.results[i][outT.name]).T for i in range(NCORES)],
        axis=0,
    )
    return out.astype(np.float32), res


def kernel(x, centers, betas):
    out, _ = _run(x, centers, betas, trace=False)
    return out


# revision 32
# speedup vs baseline: 1.0477x; 1.0477x over previous
"""Trainium2 Bass kernel for nn_Cov_EBFLayer.

Math: out[b,o] = exp(-quad[o,b]),
  quad[o,b] = diff^T P_o diff,  diff = c_o - x_b,  P_o = B_o B_o^T  (PSD Gram)
            = sum_{d,f} P[o,d,f] x_d x_f - 2 v_o^T x + q3_o,  v = P c, q3 = c^T P c

Kernel strategy (per core, batch-sharded 8 x 1024):
  Symmetric-pair feature map over cyclic offsets: unordered pairs {d, f} at
  cyclic distance k are covered once by offset-k rows (d, (d+k)%64), k=1..32.
  17 feature chunks of 128 rows x 1024 batch:
    - 16 "pair" chunks, offsets (2j+1, 2j+2) in the two 64-row halves.
      Built per-chunk via one of three paths (engine balancing):
        S: one full-width DVE tensor_mul of xb2=[x;x] against a rotated
           "slot" operand (host-precomputed rotations, loaded in one DMA)
        P: same, on the GPSIMD engine
        A: PE indicator matmul computes s = x_d + x_f into PSUM, ACT engine
           squares it: u = (x_d+x_f)^2; host adjusts W (A=coeff/2) and folds
           the unwanted x^2 cross terms into the diagonal weights.
    - 1 "misc" chunk: rows 0:64 = x_d^2 (DVE), rows 64:128 = x_d (DVE copy).
  Main contraction: per chunk, 4 accumulating matmuls (2 o-halves x 2 b-tiles
  of 512, one PSUM bank each). Epilogue: ACT Exp(scale=-1, bias=-q3) -> fp16.
Host does weight prep (P = beta beta^T, W chunk layout, v, q3; O(model)) and
layout-only data movement (x transpose + rotated copies).
"""

import sys
from contextlib import ExitStack

import numpy as np

sys.path.insert(0, "/opt/trn_rl_repo")

import concourse.bass as bass  # noqa: E402
import concourse.tile as tile  # noqa: E402
from concourse import bacc, mybir  # noqa: E402
from concourse import bass_utils  # noqa: E402
from concourse._compat import with_exitstack  # noqa: E402

B, D, O, NCORES = 8192, 64, 256, 8
BSH = B // NCORES  # 1024 per-core batch shard
BT = 512  # matmul free-dim tile (one PSUM bank of fp32)
F32 = mybir.dt.float32
F16 = mybir.dt.float16

# Accumulation-order chunk sequence. Pair chunk at list position j (skipping
# the misc entry) gets cyclic offsets (2j+1, 2j+2); paths: M=misc,
# S=slot DVE mul (products), P=slot GPSIMD mul (products),
# A=PE sum + ACT square (u-features), BD=PE sum + DVE mul by x
# (v-features: x_d^2 + x_d x_f; GPSIMD cannot read PSUM).
_PATTERN = ["M", "A", "S", "A", "P", "BD", "A", "S", "A", "P", "BD", "A", "S", "A", "P", "BD", "A"]
SEQ = []
_pj = 0
for _p in _PATTERN:
    if _p == "M":
        SEQ.append(("M", 0, 0))
    else:
        SEQ.append((_p, 2 * _pj + 1, 2 * _pj + 2))
        _pj += 1
NCH = len(SEQ)  # 17
NACT = sum(1 for s in SEQ if s[0] in ("A", "BD"))
SLOT_POS = [j for j, s in enumerate(SEQ) if s[0] in ("S", "P")]
NSLOT = len(SLOT_POS)
NSLOT_A = min(3, NSLOT)  # first slots arrive in an early DMA
W1CH = 4  # chunks in the first W transfer (so main matmuls start early)


@with_exitstack
def _kernel(ctx: ExitStack, tc, outT, xT, wts, ind, bias, xslots):
    nc = tc.nc

    cpool = ctx.enter_context(tc.tile_pool(name="const", bufs=1))
    gpool = ctx.enter_context(tc.tile_pool(name="gtiles", bufs=5))
    opool = ctx.enter_context(tc.tile_pool(name="outs", bufs=4))
    qpool = ctx.enter_context(tc.tile_pool(name="psum_q", bufs=4, space="PSUM"))
    spool = ctx.enter_context(tc.tile_pool(name="psum_s", bufs=4, space="PSUM"))

    # ---- resident inputs; order sets DMA-queue drain priority ----
    xb2 = cpool.tile([128, BSH], F16)  # [x; x] stacked
    nc.sync.dma_start(xb2[0:D, :], xT[:])
    i_sb = cpool.tile([D, NACT * 128], F16)
    nc.sync.dma_start(i_sb[:], ind[:])
    b_sb = cpool.tile([128, 2], F32)  # -q3 per o-half
    nc.sync.dma_start(b_sb[:], bias[:])
    w_sb = cpool.tile([128, NCH * O], F16)
    nc.sync.dma_start(w_sb[:, 0 : W1CH * O], wts[:, 0 : W1CH * O])
    nc.sync.dma_start(w_sb[:, W1CH * O :], wts[:, W1CH * O :])
    slots_sb = cpool.tile([128, NSLOT * BSH], F16)
    nc.sync.dma_start(
        slots_sb[:, 0 : NSLOT_A * BSH], xslots[:, 0 : NSLOT_A * BSH]
    )
    nc.sync.dma_start(slots_sb[:, NSLOT_A * BSH :], xslots[:, NSLOT_A * BSH :])

    # duplicate x rows on DVE (4x copy) instead of re-reading HBM
    g_misc = cpool.tile([128, BSH], F16)  # [x^2; x]
    nc.vector.tensor_copy(xb2[D : 2 * D, :], xb2[0:D, :])
    nc.vector.tensor_copy(g_misc[D:128, :], xb2[0:D, :])

    # PSUM bank = 2 KB/partition, so matmul outputs are [128, 512] fp32 max;
    # quad accumulates in 4 tiles (2 o-halves x 2 b-tiles).
    pq = {
        (oh, bt): qpool.tile([128, BT], F32, name=f"pq{oh}{bt}", tag="pq")
        for oh in range(2)
        for bt in range(2)
    }

    # ---- PE warm-up: a few matmuls to bridge the gap until the first G
    # chunk is ready (keeps HAM activity up). Overwritten by the real
    # accumulation. ----
    for i in range(4):
        nc.tensor.matmul(
            pq[(i % 2, (i // 2) % 2)][:],
            xb2[0:D, 0:128],
            xb2[0:D, 0:BT],
            start=True,
            stop=True,
        )

    # misc diag half: x_d^2
    nc.vector.tensor_mul(g_misc[0:D, :], xb2[0:D, :], xb2[0:D, :])

    act_pos = [j for j, s in enumerate(SEQ) if s[0] in ("A", "BD")]
    slot_of = {j: si for si, j in enumerate(SLOT_POS)}
    s_tiles = {}
    state = {"ind_ptr": 0, "squares_done": 0}

    def top_up_inds():
        # keep <=2 chunks of indicator matmuls in flight ahead of the squares
        while (
            state["ind_ptr"] < len(act_pos)
            and state["ind_ptr"] - state["squares_done"] < 2
        ):
            ai = state["ind_ptr"]
            j = act_pos[ai]
            ss = []
            for bt in range(2):
                s = spool.tile([128, BT], F32, tag="s")
                nc.tensor.matmul(
                    s[:],
                    i_sb[:, ai * 128 : (ai + 1) * 128],
                    xb2[0:D, bt * BT : (bt + 1) * BT],
                    start=True,
                    stop=True,
                )
                ss.append(s)
            s_tiles[j] = ss
            state["ind_ptr"] += 1

    # ---- main loop: build G chunk, 4 accumulating matmuls ----
    for j, (p, k1, k2) in enumerate(SEQ):
        top_up_inds()
        if p == "M":
            g = g_misc
        elif p in ("S", "P"):
            g = gpool.tile([128, BSH], F16, tag="g")
            eng = nc.vector if p == "S" else nc.gpsimd
            si = slot_of[j]
            eng.tensor_mul(
                g[:], xb2[:], slots_sb[:, si * BSH : (si + 1) * BSH]
            )
        elif p == "A":  # square the pair-sums on ACT straight out of PSUM
            g = gpool.tile([128, BSH], F16, tag="g")
            for bt in range(2):
                nc.scalar.activation(
                    g[:, bt * BT : (bt + 1) * BT],
                    s_tiles[j][bt][:],
                    mybir.ActivationFunctionType.Square,
                )
            state["squares_done"] += 1
        else:  # BD: v = s * x on DVE straight out of PSUM
            g = gpool.tile([128, BSH], F16, tag="g")
            for bt in range(2):
                nc.vector.tensor_mul(
                    g[:, bt * BT : (bt + 1) * BT],
                    s_tiles[j][bt][:],
                    xb2[:, bt * BT : (bt + 1) * BT],
                )
            state["squares_done"] += 1
        for bt in range(2):
            for oh in range(2):
                nc.tensor.matmul(
                    pq[(oh, bt)][:],
                    w_sb[:, j * O + oh * 128 : j * O + oh * 128 + 128],
                    g[:, bt * BT : (bt + 1) * BT],
                    start=(j == 0),
                    stop=(j == NCH - 1),
                )

    # ---- epilogue: out = exp(-(quad + q3)), one DMA per PSUM tile ----
    for oh in range(2):
        for bt in range(2):
            ob = opool.tile([128, BT], F16, tag="ob")
            nc.scalar.activation(
                ob[:],
                pq[(oh, bt)][:],
                mybir.ActivationFunctionType.Exp,
                bias=b_sb[:, oh : oh + 1],
                scale=-1.0,
            )
            nc.sync.dma_start(
                outT[oh * 128 : (oh + 1) * 128, bt * BT : (bt + 1) * BT], ob[:]
            )


_CACHE = {}


def _build():
    if "nc" in _CACHE:
        return _CACHE["nc"], _CACHE["aps"]
    nc = bacc.Bacc(
        "TRN2", target_bir_lowering=False, debug=False, num_devices=NCORES
    )
    xT = nc.dram_tensor("xT", [D, BSH], F16, kind="ExternalInput").ap()
    wts = nc.dram_tensor("wts", [128, NCH * O], F16, kind="ExternalInput").ap()
    ind = nc.dram_tensor("ind", [D, NACT * 128], F16, kind="ExternalInput").ap()
    bias = nc.dram_tensor("bias", [128, 2], F32, kind="ExternalInput").ap()
    xslots = nc.dram_tensor(
        "xslots", [128, NSLOT * BSH], F16, kind="ExternalInput"
    ).ap()
    outT = nc.dram_tensor("outT", [O, BSH], F16, kind="ExternalOutput").ap()
    with tile.TileContext(nc) as tc:
        _kernel(tc, outT, xT, wts, ind, bias, xslots)
    nc.compile()
    _CACHE["nc"] = nc
    _CACHE["aps"] = (xT, wts, ind, bias, xslots, outT)
    return nc, _CACHE["aps"]


def _host_prep(x, centers, betas):
    x32 = np.asarray(x, np.float32)
    betas32 = np.asarray(betas, np.float32)
    cen = np.asarray(centers, np.float32).reshape(O, D)
    # weight prep: O(model), batch-independent
    P = np.matmul(betas32, betas32.transpose(0, 2, 1))  # [O, D, D]
    w = np.einsum("ofe,of->oe", betas32, cen)
    v = np.einsum("ode,oe->od", betas32, w)
    q3 = np.einsum("oe,oe->o", w, w)

    dd = np.arange(D)
    R = np.zeros((O, D), np.float32)  # x^2 corrections from A-chunks
    Wstack = np.zeros((NCH, 128, O), np.float32)
    Istack = []
    for j, (p, k1, k2) in enumerate(SEQ):
        if p == "M":
            continue
        for half, k in ((0, k1), (1, k2)):
            f = (dd + k) % D
            coeff = (2.0 if k < D // 2 else 1.0) * P[:, dd, f]  # [O, 64]
            if p == "A":
                A_ = coeff * 0.5
                Wstack[j, half * D : (half + 1) * D, :] = A_.T
                R[:, dd] += A_
                R[:, f] += A_  # f is a permutation: indices unique
            elif p == "BD":
                Wstack[j, half * D : (half + 1) * D, :] = coeff.T
                R[:, dd] += coeff  # v = x_d^2 + x_d x_f: correct only d
            else:
                Wstack[j, half * D : (half + 1) * D, :] = coeff.T
        if p in ("A", "BD"):
            I = np.zeros((D, 128), np.float32)
            pp = np.arange(128)
            dcol = pp % D
            kcol = np.where(pp < D, k1, k2)
            I[dcol, pp] += 1.0
            I[(dcol + kcol) % D, pp] += 1.0
            Istack.append(I)
    mj = next(j for j, s in enumerate(SEQ) if s[0] == "M")
    Wstack[mj, 0:D, :] = (P[:, dd, dd] - R).T
    Wstack[mj, D:128, :] = (-2.0 * v).T

    wts = np.ascontiguousarray(
        Wstack.transpose(1, 0, 2).reshape(128, NCH * O)
    ).astype(np.float16)
    ind = np.ascontiguousarray(np.concatenate(Istack, axis=1)).astype(np.float16)
    bias = np.ascontiguousarray((-q3).reshape(2, 128).T).astype(np.float32)

    xT_shards = []
    xslot_shards = []
    for i in range(NCORES):
        xTi = np.ascontiguousarray(
            x32[i * BSH : (i + 1) * BSH].T
        ).astype(np.float16)
        xT_shards.append(xTi)
        # rotated slot operands, in consumption order (layout-only gather)
        sl = np.empty((128, NSLOT, BSH), np.float16)
        for si, j in enumerate(SLOT_POS):
            _, k1, k2 = SEQ[j]
            sl[0:D, si, :] = np.roll(xTi, -k1, axis=0)
            sl[D:128, si, :] = np.roll(xTi, -k2, axis=0)
        xslot_shards.append(
            np.ascontiguousarray(sl.reshape(128, NSLOT * BSH))
        )
    return xT_shards, xslot_shards, wts, ind, bias


def _run(x, centers, betas, trace=False):
    nc, (xT, wts_ap, ind_ap, bias_ap, xslots_ap, outT) = _build()
    xT_shards, xslot_shards, wts, ind, bias = _host_prep(x, centers, betas)
    in_maps = [
        {
            xT.name: xT_shards[i],
            wts_ap.name: wts,
            ind_ap.name: ind,
            bias_ap.name: bias,
            xslots_ap.name: xslot_shards[i],
        }
        for i in range(NCORES)
    ]
    res = bass_utils.run_bass_kernel_spmd(
        nc, in_maps, core_ids=list(range(NCORES)), trace=trace
    )
    out = np.concatenate(
        [np.asarray(res.results[i][outT.name]).T for i in range(NCORES)],
        axis=0,
    )
    return out.astype(np.float32), res


def kernel(x, centers, betas):
    out, _ = _run(x, centers, betas, trace=False)
    return out
